# revision 23
# baseline (speedup 1.0000x reference)
"""Trainium2 Bass kernel for conv->BN->ReLU->1x1->ReLU->1x1->segment-mean classifier.

Contract: kernel(**inputs) takes FULL inputs (as from setup_inputs()) and
returns the FULL [4,16,512,512] float32 output. Internally shards across 8
NeuronCores: core = sample*2 + row_half (data-parallel over batch and H).

Key insights:
- The final nearest-neighbor upsample only reads pooled segment s = 2h, i.e.
  only segments with EVEN block-column index -> compute only those 256 of 512
  pixel columns per row (half the work).
- relu(ph + b2) = max(ph, -b2) + b2, and the +b2 sums to a constant per
  segment -> the whole h bias+relu+accumulate stage is ONE fused DVE
  scalar_tensor_tensor per tile; b2 is re-added on the host.
- Instruction emission is software-pipelined (h-stage of iter i-1 emitted
  after conv of iter i) so the PE queue never drains and the tensor engine
  ramps to its full-speed p-state.
- Block-sum reduction runs on the otherwise-idle GPSIMD engine.

Device per core (half-sample, 256 rows x 16 even blocks x 16 cols):
  feat = relu(conv3x3(x, w1*bn_inv) + bn_bias)   K=28 im2col matmul (host im2col)
  ph   = w2 @ feat                               K=256 matmul
  hacc += max(ph, -b2)                           fused relu+acc
  pooled[128ch, 16t x 16blk] = block sums of hacc (gpsimd tree reduce)
Host applies: /256 + b2, w3 @ . + b3, then broadcast rows/cols.
"""
import sys, types
sys.path.insert(0, '/opt/trn_rl_repo')

import numpy as np
import ml_dtypes
from contextlib import ExitStack

import concourse.bass as bass
import concourse.tile as tile
from concourse import bacc, mybir
from concourse.alu_op_type import AluOpType
from concourse.bass_utils import run_bass_kernel_spmd

EPS = 1e-5
N_CORES = 8
MM_DT = mybir.dt.bfloat16

# how many of the 1024 conv-relu elems per iter the DVE takes (rest on ACT)
DVE_RELU = 128


def _build_program():
    nc = bacc.Bacc("TRN2", num_devices=N_CORES, debug=False,
                   target_bir_lowering=False)
    f32 = mybir.dt.float32
    # host-prebuilt im2col patches: [64 part, t(16) x r(16) x m(16) x j(16)]
    patch_d = nc.dram_tensor("patch", [64, 65536], MM_DT, kind="ExternalInput")
    w1t = nc.dram_tensor("w1t", [64, 128], MM_DT, kind="ExternalInput")
    w2t = nc.dram_tensor("w2t", [128, 256], MM_DT, kind="ExternalInput")
    nb2 = nc.dram_tensor("nb2", [128, 1], f32, kind="ExternalInput")
    pooled = nc.dram_tensor("pooled", [128, 256], f32, kind="ExternalOutput")

    Relu = mybir.ActivationFunctionType.Relu
    add, mx = AluOpType.add, AluOpType.max

    with tile.TileContext(nc) as tc:
        with ExitStack() as ctx:
            consts = ctx.enter_context(tc.tile_pool(name="consts", bufs=1))
            patch_pool = ctx.enter_context(tc.tile_pool(name="patch", bufs=3))
            pc_pool = ctx.enter_context(
                tc.tile_pool(name="pc", bufs=2, space="PSUM"))
            feat_pool = ctx.enter_context(tc.tile_pool(name="feat", bufs=3))
            ph_pool = ctx.enter_context(
                tc.tile_pool(name="ph", bufs=3, space="PSUM"))
            hacc_pool = ctx.enter_context(tc.tile_pool(name="hacc", bufs=2))
            red_pool = ctx.enter_context(tc.tile_pool(name="red", bufs=2))

            w1t_sb = consts.tile([64, 128], MM_DT)
            nc.sync.dma_start(w1t_sb[:], w1t.ap())
            w2t_sb = consts.tile([128, 256], MM_DT)
            nc.sync.dma_start(w2t_sb[:], w2t.ap())
            nb2_sb = consts.tile([128, 1], f32)
            nc.sync.dma_start(nb2_sb[:], nb2.ap())
            pooled_sb = consts.tile([128, 256], f32)

            state = {"hacc": None}

            def emit_h(feat, t, j):
                ph = ph_pool.tile([128, 512], f32)  # 1 bank
                nc.tensor.matmul(ph[:], w2t_sb[:, 0:128], feat[:, 0:512],
                                 start=True, stop=False)
                nc.tensor.matmul(ph[:], w2t_sb[:, 128:256],
                                 feat[:, 512:1024], start=False, stop=True)
                if j == 0:
                    hacc_new = hacc_pool.tile([128, 512], f32)
                    state["hacc"] = hacc_new
                    nc.vector.tensor_scalar(state["hacc"][:], ph[:],
                                            nb2_sb[:], None, mx)
                else:
                    nc.vector.scalar_tensor_tensor(state["hacc"][:], ph[:],
                                                   nb2_sb[:],
                                                   state["hacc"][:], mx, add)
                if j == 7:
                    # block sums on gpsimd: hacc cols = (rr 2, m 16, px 16)
                    hacc = state["hacc"]
                    t1 = red_pool.tile([128, 256], f32)
                    nc.gpsimd.tensor_tensor(t1[:], hacc[:, 0:256],
                                            hacc[:, 256:512], add)
                    v = t1[:].rearrange("p (m px) -> p m px", px=16)
                    for half in (8, 4, 2, 1):
                        nc.gpsimd.tensor_tensor(v[:, :, 0:half],
                                                v[:, :, 0:half],
                                                v[:, :, half:2 * half], add)
                    nc.gpsimd.tensor_copy(
                        pooled_sb[:, t * 16:(t + 1) * 16], v[:, :, 0])

            pap = patch_d.ap()
            pending = []  # iters awaiting h-stage emission (2-iter skew)
            for t in range(16):
                patch = patch_pool.tile([64, 4096], MM_DT)
                nc.sync.dma_start(patch[:], pap[:, t * 4096:(t + 1) * 4096])
                for j in range(8):  # 2 rows per iter
                    c0 = j * 512
                    pc = pc_pool.tile([128, 1024], f32)  # 2 banks
                    nc.tensor.matmul(pc[:, 0:512], w1t_sb[0:28, :],
                                     patch[0:28, c0:c0 + 512],
                                     start=True, stop=True,
                                     tile_position=(0, 0))
                    nc.tensor.matmul(pc[:, 512:1024], w1t_sb[32:60, :],
                                     patch[32:60, c0:c0 + 512],
                                     start=True, stop=True,
                                     tile_position=(32, 0))
                    feat = feat_pool.tile([128, 1024], MM_DT)
                    s = 1024 - DVE_RELU
                    nc.scalar.activation(feat[:, 0:s], pc[:, 0:s], Relu)
                    if DVE_RELU:
                        nc.vector.tensor_scalar(feat[:, s:1024],
                                                pc[:, s:1024], 0.0, None, mx)
                    if pending:
                        emit_h(*pending.pop(0))
                    pending.append((feat, t, j))
            while pending:
                emit_h(*pending.pop(0))
            nc.sync.dma_start(pooled.ap(), pooled_sb[:])
    nc.compile()
    return nc


_NC_CACHE = None


def _get_program():
    global _NC_CACHE
    if _NC_CACHE is None:
        _NC_CACHE = _build_program()
    return _NC_CACHE


def _run_device(in_maps, trace=False):
    nc = _get_program()
    if trace:
        import trn_agent_boot.trn_boot as _tb
        _hook = _tb._ntff_profile_via_ctypes('/opt/axon/libaxon_pjrt.so')
        _m = types.ModuleType('antenv.axon_hooks')
        _m.get_axon_ntff_profile_hook = lambda: _hook
        sys.modules['antenv.axon_hooks'] = _m
    return run_bass_kernel_spmd(nc, in_maps, list(range(N_CORES)), trace=trace)


def _prep_inputs(x, w1, b1, bn_gamma, bn_beta, bn_mean, bn_var, w2, b2):
    x = np.asarray(x, np.float32)
    inv = (bn_gamma / np.sqrt(bn_var + EPS)).astype(np.float32)
    w1f = (np.asarray(w1, np.float32) * inv[:, None, None, None])
    bias1 = (b1 * inv + bn_beta - bn_mean * inv).astype(np.float32)

    w1t_np = np.zeros((64, 128), np.float32)
    for dy in range(3):
        for dx in range(3):
            for ci in range(3):
                k = (dy * 3 + dx) * 3 + ci
                w1t_np[k, :] = w1f[0:128, ci, dy, dx]
                w1t_np[32 + k, :] = w1f[128:256, ci, dy, dx]
    w1t_np[27, :] = bias1[0:128]
    w1t_np[59, :] = bias1[128:256]

    w2t_np = np.zeros((128, 256), np.float32)
    w2t_np[:, 0:128] = np.asarray(w2, np.float32)[:, 0:128].T
    w2t_np[:, 128:256] = np.asarray(w2, np.float32)[:, 128:256].T
    nb2_np = (-np.asarray(b2, np.float32)).reshape(128, 1)

    w1t_bf = w1t_np.astype(ml_dtypes.bfloat16)
    w2t_bf = w2t_np.astype(ml_dtypes.bfloat16)
    xp = np.pad(x, ((0, 0), (0, 0), (1, 1), (1, 1)))  # [4,3,514,514]
    # even-block column gather: block m covers x cols 32m+j+dx-1 (padded idx
    # 32m+j+dx), j in 0..15
    colbase = (32 * np.arange(16)[:, None] + np.arange(16)[None, :]).ravel()
    in_maps = []
    for core in range(N_CORES):
        b, half = core // 2, core % 2
        patch_np = np.zeros((64, 65536), ml_dtypes.bfloat16)
        for dy in range(3):
            rows = xp[b, :, half * 256 + dy:half * 256 + dy + 256, :]
            for dx in range(3):
                sub = rows[:, :, colbase + dx]  # [3, 256, 256]
                sub = sub.reshape(3, 256 * 256).astype(ml_dtypes.bfloat16)
                k = (dy * 3 + dx) * 3
                patch_np[k:k + 3] = sub
                patch_np[32 + k:32 + k + 3] = sub
        patch_np[27] = 1.0
        patch_np[59] = 1.0
        in_maps.append({"patch": patch_np, "w1t": w1t_bf,
                        "w2t": w2t_bf, "nb2": nb2_np})
    return in_maps


def _postprocess(results, b2, w3, b3):
    b2 = np.asarray(b2, np.float32)
    w3 = np.asarray(w3, np.float32)
    b3 = np.asarray(b3, np.float32)
    out = np.empty((4, 16, 512, 512), np.float32)
    for b in range(4):
        hs = np.concatenate(
            [results[2 * b]["pooled"], results[2 * b + 1]["pooled"]], axis=1)
        # hs[c, q*16+m] = sum over block (q, 2m) of max(ph, -b2)
        pooled_mean = hs / 256.0 + b2[:, None]  # [128, 512]
        logits = w3 @ pooled_mean + b3[:, None]  # [16, 512]; col = h
        out[b] = np.broadcast_to(logits[:, :, None], (16, 512, 512))
    return out


def kernel(x, w1, b1, bn_gamma, bn_beta, bn_mean, bn_var, w2, b2, w3, b3,
           _trace=False):
    in_maps = _prep_inputs(x, w1, b1, bn_gamma, bn_beta, bn_mean, bn_var,
                           w2, b2)
    res = _run_device(in_maps, trace=_trace)
    out = _postprocess(res.results, b2, w3, b3)
    if _trace:
        kernel.last_exec_time_ns = res.exec_time_ns
        kernel.last_results = res
    return out


# revision 24
# speedup vs baseline: 1.5275x; 1.5275x over previous
"""Trainium2 Bass kernel for conv->BN->ReLU->1x1->ReLU->1x1->segment-mean classifier.

Contract: kernel(**inputs) takes FULL inputs (as from setup_inputs()) and
returns the FULL [4,16,512,512] float32 output. Internally shards across 8
NeuronCores: core = sample*2 + row_half (data-parallel over batch and H).

Key insights:
- The final nearest-neighbor upsample only reads pooled segment s = 2h, i.e.
  only segments with EVEN block-column index -> compute only those 256 of 512
  pixel columns per row (half the work).
- relu(ph + b2) = max(ph, -b2) + b2, and the +b2 sums to a constant per
  segment -> the whole h bias+relu+accumulate stage is ONE fused DVE
  scalar_tensor_tensor per tile; b2 is re-added on the host.
- Instruction emission is software-pipelined (h-stage of iter i-1 emitted
  after conv of iter i) so the PE queue never drains and the tensor engine
  ramps to its full-speed p-state.
- Block-sum reduction runs on the otherwise-idle GPSIMD engine.

Device per core (half-sample, 256 rows x 16 even blocks x 16 cols):
  feat = relu(conv3x3(x, w1*bn_inv) + bn_bias)   K=28 im2col matmul (host im2col)
  ph   = w2 @ feat                               K=256 matmul
  hacc += max(ph, -b2)                           fused relu+acc
  pooled[128ch, 16t x 16blk] = block sums of hacc (gpsimd tree reduce)
Host applies: /256 + b2, w3 @ . + b3, then broadcast rows/cols.
"""
import sys, types
sys.path.insert(0, '/opt/trn_rl_repo')

import numpy as np
import ml_dtypes
from contextlib import ExitStack

import concourse.bass as bass
import concourse.tile as tile
from concourse import bacc, mybir
from concourse.alu_op_type import AluOpType
from concourse.bass_utils import run_bass_kernel_spmd

EPS = 1e-5
N_CORES = 8
MM_DT = mybir.dt.bfloat16

# how many of the 1024 conv-relu elems per iter the DVE takes (rest on ACT)
DVE_RELU = 128


def _build_program():
    nc = bacc.Bacc("TRN2", num_devices=N_CORES, debug=False,
                   target_bir_lowering=False)
    f32 = mybir.dt.float32
    # host-prebuilt im2col patches: [64 part, t(16) x r(16) x m(16) x j(16)]
    patch_d = nc.dram_tensor("patch", [64, 65536], MM_DT, kind="ExternalInput")
    w1t = nc.dram_tensor("w1t", [64, 128], MM_DT, kind="ExternalInput")
    w2t = nc.dram_tensor("w2t", [128, 256], MM_DT, kind="ExternalInput")
    nb2 = nc.dram_tensor("nb2", [128, 1], f32, kind="ExternalInput")
    pooled = nc.dram_tensor("pooled", [128, 256], f32, kind="ExternalOutput")

    Relu = mybir.ActivationFunctionType.Relu
    add, mx = AluOpType.add, AluOpType.max

    with tile.TileContext(nc) as tc:
        with ExitStack() as ctx:
            consts = ctx.enter_context(tc.tile_pool(name="consts", bufs=1))
            patch_pool = ctx.enter_context(tc.tile_pool(name="patch", bufs=3))
            pc_pool = ctx.enter_context(
                tc.tile_pool(name="pc", bufs=3, space="PSUM"))
            feat_pool = ctx.enter_context(tc.tile_pool(name="feat", bufs=3))
            ph_pool = ctx.enter_context(
                tc.tile_pool(name="ph", bufs=2, space="PSUM"))
            hacc_pool = ctx.enter_context(tc.tile_pool(name="hacc", bufs=2))
            red_pool = ctx.enter_context(tc.tile_pool(name="red", bufs=2))

            w1t_sb = consts.tile([64, 128], MM_DT)
            nc.sync.dma_start(w1t_sb[:], w1t.ap())
            w2t_sb = consts.tile([128, 256], MM_DT)
            nc.sync.dma_start(w2t_sb[:], w2t.ap())
            nb2_sb = consts.tile([128, 1], f32)
            nc.sync.dma_start(nb2_sb[:], nb2.ap())
            pooled_sb = consts.tile([128, 256], f32)

            state = {"hacc": None}

            def emit_h(feat, t, j):
                ph = ph_pool.tile([128, 512], f32)  # 1 bank
                nc.tensor.matmul(ph[:], w2t_sb[:, 0:128], feat[:, 0:512],
                                 start=True, stop=False)
                nc.tensor.matmul(ph[:], w2t_sb[:, 128:256],
                                 feat[:, 512:1024], start=False, stop=True)
                if j == 0:
                    hacc_new = hacc_pool.tile([128, 512], f32)
                    state["hacc"] = hacc_new
                    nc.vector.tensor_scalar(state["hacc"][:], ph[:],
                                            nb2_sb[:], None, mx)
                else:
                    nc.vector.scalar_tensor_tensor(state["hacc"][:], ph[:],
                                                   nb2_sb[:],
                                                   state["hacc"][:], mx, add)
                if j == 7:
                    # block sums on gpsimd: hacc cols = (rr 2, m 16, px 16)
                    hacc = state["hacc"]
                    t1 = red_pool.tile([128, 256], f32)
                    nc.gpsimd.tensor_tensor(t1[:], hacc[:, 0:256],
                                            hacc[:, 256:512], add)
                    v = t1[:].rearrange("p (m px) -> p m px", px=16)
                    for half in (8, 4, 2, 1):
                        nc.gpsimd.tensor_tensor(v[:, :, 0:half],
                                                v[:, :, 0:half],
                                                v[:, :, half:2 * half], add)
                    nc.gpsimd.tensor_copy(
                        pooled_sb[:, t * 16:(t + 1) * 16], v[:, :, 0])

            pap = patch_d.ap()
            pending = []  # iters awaiting h-stage emission (2-iter skew)
            for t in range(16):
                patch = patch_pool.tile([64, 4096], MM_DT)
                nc.sync.dma_start(patch[:], pap[:, t * 4096:(t + 1) * 4096])
                for j in range(8):  # 2 rows per iter
                    c0 = j * 512
                    pc = pc_pool.tile([128, 1024], f32)  # 2 banks
                    nc.tensor.matmul(pc[:, 0:512], w1t_sb[0:28, :],
                                     patch[0:28, c0:c0 + 512],
                                     start=True, stop=True,
                                     tile_position=(0, 0))
                    nc.tensor.matmul(pc[:, 512:1024], w1t_sb[32:60, :],
                                     patch[32:60, c0:c0 + 512],
                                     start=True, stop=True,
                                     tile_position=(32, 0))
                    feat = feat_pool.tile([128, 1024], MM_DT)
                    s = 1024 - DVE_RELU
                    nc.scalar.activation(feat[:, 0:s], pc[:, 0:s], Relu)
                    if DVE_RELU:
                        nc.vector.tensor_scalar(feat[:, s:1024],
                                                pc[:, s:1024], 0.0, None, mx)
                    if pending:
                        emit_h(*pending.pop(0))
                    pending.append((feat, t, j))
            while pending:
                emit_h(*pending.pop(0))
            nc.sync.dma_start(pooled.ap(), pooled_sb[:])
    nc.compile()
    return nc


_NC_CACHE = None


def _get_program():
    global _NC_CACHE
    if _NC_CACHE is None:
        _NC_CACHE = _build_program()
    return _NC_CACHE


def _run_device(in_maps, trace=False):
    nc = _get_program()
    if trace:
        import trn_agent_boot.trn_boot as _tb
        _hook = _tb._ntff_profile_via_ctypes('/opt/axon/libaxon_pjrt.so')
        _m = types.ModuleType('antenv.axon_hooks')
        _m.get_axon_ntff_profile_hook = lambda: _hook
        sys.modules['antenv.axon_hooks'] = _m
    return run_bass_kernel_spmd(nc, in_maps, list(range(N_CORES)), trace=trace)


def _prep_inputs(x, w1, b1, bn_gamma, bn_beta, bn_mean, bn_var, w2, b2):
    x = np.asarray(x, np.float32)
    inv = (bn_gamma / np.sqrt(bn_var + EPS)).astype(np.float32)
    w1f = (np.asarray(w1, np.float32) * inv[:, None, None, None])
    bias1 = (b1 * inv + bn_beta - bn_mean * inv).astype(np.float32)

    w1t_np = np.zeros((64, 128), np.float32)
    for dy in range(3):
        for dx in range(3):
            for ci in range(3):
                k = (dy * 3 + dx) * 3 + ci
                w1t_np[k, :] = w1f[0:128, ci, dy, dx]
                w1t_np[32 + k, :] = w1f[128:256, ci, dy, dx]
    w1t_np[27, :] = bias1[0:128]
    w1t_np[59, :] = bias1[128:256]

    w2t_np = np.zeros((128, 256), np.float32)
    w2t_np[:, 0:128] = np.asarray(w2, np.float32)[:, 0:128].T
    w2t_np[:, 128:256] = np.asarray(w2, np.float32)[:, 128:256].T
    nb2_np = (-np.asarray(b2, np.float32)).reshape(128, 1)

    w1t_bf = w1t_np.astype(ml_dtypes.bfloat16)
    w2t_bf = w2t_np.astype(ml_dtypes.bfloat16)
    xp = np.pad(x, ((0, 0), (0, 0), (1, 1), (1, 1)))  # [4,3,514,514]
    # even-block column gather: block m covers x cols 32m+j+dx-1 (padded idx
    # 32m+j+dx), j in 0..15
    colbase = (32 * np.arange(16)[:, None] + np.arange(16)[None, :]).ravel()
    in_maps = []
    for core in range(N_CORES):
        b, half = core // 2, core % 2
        patch_np = np.zeros((64, 65536), ml_dtypes.bfloat16)
        for dy in range(3):
            rows = xp[b, :, half * 256 + dy:half * 256 + dy + 256, :]
            for dx in range(3):
                sub = rows[:, :, colbase + dx]  # [3, 256, 256]
                sub = sub.reshape(3, 256 * 256).astype(ml_dtypes.bfloat16)
                k = (dy * 3 + dx) * 3
                patch_np[k:k + 3] = sub
                patch_np[32 + k:32 + k + 3] = sub
        patch_np[27] = 1.0
        patch_np[59] = 1.0
        in_maps.append({"patch": patch_np, "w1t": w1t_bf,
                        "w2t": w2t_bf, "nb2": nb2_np})
    return in_maps


def _postprocess(results, b2, w3, b3):
    b2 = np.asarray(b2, np.float32)
    w3 = np.asarray(w3, np.float32)
    b3 = np.asarray(b3, np.float32)
    out = np.empty((4, 16, 512, 512), np.float32)
    for b in range(4):
        hs = np.concatenate(
            [results[2 * b]["pooled"], results[2 * b + 1]["pooled"]], axis=1)
        # hs[c, q*16+m] = sum over block (q, 2m) of max(ph, -b2)
        pooled_mean = hs / 256.0 + b2[:, None]  # [128, 512]
        logits = w3 @ pooled_mean + b3[:, None]  # [16, 512]; col = h
        out[b] = np.broadcast_to(logits[:, :, None], (16, 512, 512))
    return out


def kernel(x, w1, b1, bn_gamma, bn_beta, bn_mean, bn_var, w2, b2, w3, b3,
           _trace=False):
    in_maps = _prep_inputs(x, w1, b1, bn_gamma, bn_beta, bn_mean, bn_var,
                           w2, b2)
    res = _run_device(in_maps, trace=_trace)
    out = _postprocess(res.results, b2, w3, b3)
    if _trace:
        kernel.last_exec_time_ns = res.exec_time_ns
        kernel.last_results = res
    return out


# revision 25
# speedup vs baseline: 1.5300x; 1.0017x over previous
"""Trainium2 Bass kernel for conv->BN->ReLU->1x1->ReLU->1x1->segment-mean classifier.

Contract: kernel(**inputs) takes FULL inputs (as from setup_inputs()) and
returns the FULL [4,16,512,512] float32 output. Internally shards across 8
NeuronCores: core = sample*2 + row_half (data-parallel over batch and H).

Key insights:
- The final nearest-neighbor upsample only reads pooled segment s = 2h, i.e.
  only segments with EVEN block-column index -> compute only those 256 of 512
  pixel columns per row (half the work).
- relu(ph + b2) = max(ph, -b2) + b2, and the +b2 sums to a constant per
  segment -> the whole h bias+relu+accumulate stage is ONE fused DVE
  scalar_tensor_tensor per tile; b2 is re-added on the host.
- Instruction emission is software-pipelined (h-stage of iter i-1 emitted
  after conv of iter i) so the PE queue never drains and the tensor engine
  ramps to its full-speed p-state.
- Block-sum reduction runs on the otherwise-idle GPSIMD engine.

Device per core (half-sample, 256 rows x 16 even blocks x 16 cols):
  feat = relu(conv3x3(x, w1*bn_inv) + bn_bias)   K=28 im2col matmul (host im2col)
  ph   = w2 @ feat                               K=256 matmul
  hacc += max(ph, -b2)                           fused relu+acc
  pooled[128ch, 16t x 16blk] = block sums of hacc (gpsimd tree reduce)
Host applies: /256 + b2, w3 @ . + b3, then broadcast rows/cols.
"""
import sys, types
sys.path.insert(0, '/opt/trn_rl_repo')

import numpy as np
import ml_dtypes
from contextlib import ExitStack

import concourse.bass as bass
import concourse.tile as tile
from concourse import bacc, mybir
from concourse.alu_op_type import AluOpType
from concourse.bass_utils import run_bass_kernel_spmd

EPS = 1e-5
N_CORES = 8
MM_DT = mybir.dt.bfloat16

# how many of the 1024 conv-relu elems per iter the DVE takes (rest on ACT)
DVE_RELU = 128


def _build_program():
    nc = bacc.Bacc("TRN2", num_devices=N_CORES, debug=False,
                   target_bir_lowering=False)
    f32 = mybir.dt.float32
    # host-prebuilt im2col patches: [64 part, t(16) x r(16) x m(16) x j(16)]
    patch_d = nc.dram_tensor("patch", [64, 65536], MM_DT, kind="ExternalInput")
    w1t = nc.dram_tensor("w1t", [64, 128], MM_DT, kind="ExternalInput")
    w2t = nc.dram_tensor("w2t", [128, 256], MM_DT, kind="ExternalInput")
    nb2 = nc.dram_tensor("nb2", [128, 1], f32, kind="ExternalInput")
    pooled = nc.dram_tensor("pooled", [128, 256], f32, kind="ExternalOutput")

    Relu = mybir.ActivationFunctionType.Relu
    add, mx = AluOpType.add, AluOpType.max

    with tile.TileContext(nc) as tc:
        with ExitStack() as ctx:
            consts = ctx.enter_context(tc.tile_pool(name="consts", bufs=1))
            patch_pool = ctx.enter_context(tc.tile_pool(name="patch", bufs=3))
            pc_pool = ctx.enter_context(
                tc.tile_pool(name="pc", bufs=3, space="PSUM"))
            feat_pool = ctx.enter_context(tc.tile_pool(name="feat", bufs=3))
            ph_pool = ctx.enter_context(
                tc.tile_pool(name="ph", bufs=2, space="PSUM"))
            hacc_pool = ctx.enter_context(tc.tile_pool(name="hacc", bufs=3))
            red_pool = ctx.enter_context(tc.tile_pool(name="red", bufs=3))

            w1t_sb = consts.tile([64, 128], MM_DT)
            nc.sync.dma_start(w1t_sb[:], w1t.ap())
            w2t_sb = consts.tile([128, 256], MM_DT)
            nc.sync.dma_start(w2t_sb[:], w2t.ap())
            nb2_sb = consts.tile([128, 1], f32)
            nc.sync.dma_start(nb2_sb[:], nb2.ap())
            pooled_sb = consts.tile([128, 256], f32)

            state = {"hacc": None}

            def emit_h(feat, t, j):
                ph = ph_pool.tile([128, 512], f32)  # 1 bank
                nc.tensor.matmul(ph[:], w2t_sb[:, 0:128], feat[:, 0:512],
                                 start=True, stop=False)
                nc.tensor.matmul(ph[:], w2t_sb[:, 128:256],
                                 feat[:, 512:1024], start=False, stop=True)
                if j == 0:
                    hacc_new = hacc_pool.tile([128, 512], f32)
                    state["hacc"] = hacc_new
                    nc.vector.tensor_scalar(state["hacc"][:], ph[:],
                                            nb2_sb[:], None, mx)
                else:
                    nc.vector.scalar_tensor_tensor(state["hacc"][:], ph[:],
                                                   nb2_sb[:],
                                                   state["hacc"][:], mx, add)
                if j == 7:
                    # block sums on gpsimd: hacc cols = (rr 2, m 16, px 16)
                    hacc = state["hacc"]
                    t1 = red_pool.tile([128, 256], f32)
                    nc.gpsimd.tensor_tensor(t1[:], hacc[:, 0:256],
                                            hacc[:, 256:512], add)
                    v = t1[:].rearrange("p (m px) -> p m px", px=16)
                    for half in (8, 4, 2, 1):
                        nc.gpsimd.tensor_tensor(v[:, :, 0:half],
                                                v[:, :, 0:half],
                                                v[:, :, half:2 * half], add)
                    nc.gpsimd.tensor_copy(
                        pooled_sb[:, t * 16:(t + 1) * 16], v[:, :, 0])

            pap = patch_d.ap()
            pending = []  # iters awaiting h-stage emission (2-iter skew)
            for t in range(16):
                patch = patch_pool.tile([64, 4096], MM_DT)
                nc.sync.dma_start(patch[:], pap[:, t * 4096:(t + 1) * 4096])
                for j in range(8):  # 2 rows per iter
                    c0 = j * 512
                    pc = pc_pool.tile([128, 1024], f32)  # 2 banks
                    nc.tensor.matmul(pc[:, 0:512], w1t_sb[0:28, :],
                                     patch[0:28, c0:c0 + 512],
                                     start=True, stop=True,
                                     tile_position=(0, 0))
                    nc.tensor.matmul(pc[:, 512:1024], w1t_sb[32:60, :],
                                     patch[32:60, c0:c0 + 512],
                                     start=True, stop=True,
                                     tile_position=(32, 0))
                    feat = feat_pool.tile([128, 1024], MM_DT)
                    s = 1024 - DVE_RELU
                    nc.scalar.activation(feat[:, 0:s], pc[:, 0:s], Relu)
                    if DVE_RELU:
                        nc.vector.tensor_scalar(feat[:, s:1024],
                                                pc[:, s:1024], 0.0, None, mx)
                    if pending:
                        emit_h(*pending.pop(0))
                    pending.append((feat, t, j))
            while pending:
                emit_h(*pending.pop(0))
            nc.sync.dma_start(pooled.ap(), pooled_sb[:])
    nc.compile()
    return nc


_NC_CACHE = None


def _get_program():
    global _NC_CACHE
    if _NC_CACHE is None:
        _NC_CACHE = _build_program()
    return _NC_CACHE


def _run_device(in_maps, trace=False):
    nc = _get_program()
    if trace:
        import trn_agent_boot.trn_boot as _tb
        _hook = _tb._ntff_profile_via_ctypes('/opt/axon/libaxon_pjrt.so')
        _m = types.ModuleType('antenv.axon_hooks')
        _m.get_axon_ntff_profile_hook = lambda: _hook
        sys.modules['antenv.axon_hooks'] = _m
    return run_bass_kernel_spmd(nc, in_maps, list(range(N_CORES)), trace=trace)


def _prep_inputs(x, w1, b1, bn_gamma, bn_beta, bn_mean, bn_var, w2, b2):
    x = np.asarray(x, np.float32)
    inv = (bn_gamma / np.sqrt(bn_var + EPS)).astype(np.float32)
    w1f = (np.asarray(w1, np.float32) * inv[:, None, None, None])
    bias1 = (b1 * inv + bn_beta - bn_mean * inv).astype(np.float32)

    w1t_np = np.zeros((64, 128), np.float32)
    for dy in range(3):
        for dx in range(3):
            for ci in range(3):
                k = (dy * 3 + dx) * 3 + ci
                w1t_np[k, :] = w1f[0:128, ci, dy, dx]
                w1t_np[32 + k, :] = w1f[128:256, ci, dy, dx]
    w1t_np[27, :] = bias1[0:128]
    w1t_np[59, :] = bias1[128:256]

    w2t_np = np.zeros((128, 256), np.float32)
    w2t_np[:, 0:128] = np.asarray(w2, np.float32)[:, 0:128].T
    w2t_np[:, 128:256] = np.asarray(w2, np.float32)[:, 128:256].T
    nb2_np = (-np.asarray(b2, np.float32)).reshape(128, 1)

    w1t_bf = w1t_np.astype(ml_dtypes.bfloat16)
    w2t_bf = w2t_np.astype(ml_dtypes.bfloat16)
    xp = np.pad(x, ((0, 0), (0, 0), (1, 1), (1, 1)))  # [4,3,514,514]
    # even-block column gather: block m covers x cols 32m+j+dx-1 (padded idx
    # 32m+j+dx), j in 0..15
    colbase = (32 * np.arange(16)[:, None] + np.arange(16)[None, :]).ravel()
    in_maps = []
    for core in range(N_CORES):
        b, half = core // 2, core % 2
        patch_np = np.zeros((64, 65536), ml_dtypes.bfloat16)
        for dy in range(3):
            rows = xp[b, :, half * 256 + dy:half * 256 + dy + 256, :]
            for dx in range(3):
                sub = rows[:, :, colbase + dx]  # [3, 256, 256]
                sub = sub.reshape(3, 256 * 256).astype(ml_dtypes.bfloat16)
                k = (dy * 3 + dx) * 3
                patch_np[k:k + 3] = sub
                patch_np[32 + k:32 + k + 3] = sub
        patch_np[27] = 1.0
        patch_np[59] = 1.0
        in_maps.append({"patch": patch_np, "w1t": w1t_bf,
                        "w2t": w2t_bf, "nb2": nb2_np})
    return in_maps


def _postprocess(results, b2, w3, b3):
    b2 = np.asarray(b2, np.float32)
    w3 = np.asarray(w3, np.float32)
    b3 = np.asarray(b3, np.float32)
    out = np.empty((4, 16, 512, 512), np.float32)
    for b in range(4):
        hs = np.concatenate(
            [results[2 * b]["pooled"], results[2 * b + 1]["pooled"]], axis=1)
        # hs[c, q*16+m] = sum over block (q, 2m) of max(ph, -b2)
        pooled_mean = hs / 256.0 + b2[:, None]  # [128, 512]
        logits = w3 @ pooled_mean + b3[:, None]  # [16, 512]; col = h
        out[b] = np.broadcast_to(logits[:, :, None], (16, 512, 512))
    return out


def kernel(x, w1, b1, bn_gamma, bn_beta, bn_mean, bn_var, w2, b2, w3, b3,
           _trace=False):
    in_maps = _prep_inputs(x, w1, b1, bn_gamma, bn_beta, bn_mean, bn_var,
                           w2, b2)
    res = _run_device(in_maps, trace=_trace)
    out = _postprocess(res.results, b2, w3, b3)
    if _trace:
        kernel.last_exec_time_ns = res.exec_time_ns
        kernel.last_results = res
    return out
